# revision 4
# baseline (speedup 1.0000x reference)
"""Fuzzy-antecedent kernel: out[i, r] = prod_j m_j[i, ri[r, j]] on 8 TRN2 cores.

r = i0*625 + i1*125 + i2*25 + i3*5 + i4 (lexicographic meshgrid over 5 sets
of 5), so each output row is the Kronecker product of the five 5-element
membership rows. Data-parallel over the sample axis: 16384 rows -> 2048 per
core -> 16 partition-tiles of 128. Per tile the product chain is built with
widths 25 -> 125 -> 625 -> 3125 via tensor_scalar multiplies (per-partition
scalar operand), final stage split across the DVE and ACT engines. The
output write (25.6 MB/core) is the roofline.
"""

import numpy as np

import concourse.bass as bass
import concourse.tile as tile
from concourse import bacc, mybir
from concourse.bass_utils import run_bass_kernel_spmd

N = 16384
N_CORES = 8
NPC = N // N_CORES  # 2048 rows per core
NT = NPC // 128  # 16 partition tiles per core
R = 3125
F32 = mybir.dt.float32

# Even-width padding: fp32 tensor_scalar on DVE hits 2x_2P mode only for
# even innermost dims, so padded segments are written one element wide into
# the next segment (ascending order self-heals) and tiles get a pad column.


def build_bass():
    nc = bacc.Bacc()
    # mcat[p, t*25 + j*5 + k] = m_j[t*128 + p, k] (host pre-packed)
    mcat = nc.declare_dram_parameter("mcat", [128, NT * 25], F32, isOutput=False)
    out = nc.declare_dram_parameter("out", [NPC, R], F32, isOutput=True)

    with tile.TileContext(nc) as tc:
        with (
            tc.tile_pool(name="inp", bufs=1) as ipool,
            tc.tile_pool(name="mid", bufs=2) as mpool,
            tc.tile_pool(name="obuf", bufs=4) as opool,
        ):
            mt = ipool.tile([128, NT * 25 + 2], F32)
            nc.vector.memset(mt[:, NT * 25 : NT * 25 + 2], 0.0)
            nc.sync.dma_start(out=mt[:, : NT * 25], in_=mcat[:])

            for t in range(NT):
                b = t * 25  # m0 at b, m1 at b+5, m2 at b+10, m3 at b+15, m4 at b+20
                s2 = mpool.tile([128, 26], F32, tag="s2")
                s3 = mpool.tile([128, 126], F32, tag="s3")
                s4 = mpool.tile([128, 626], F32, tag="s4")
                ot = opool.tile([128, R + 1], F32, tag="ot")

                # s2 = m3 (x) m4
                for i in range(5):
                    nc.vector.tensor_scalar_mul(
                        s2[:, i * 5 : i * 5 + 6],
                        mt[:, b + 20 : b + 26],
                        mt[:, b + 15 + i : b + 16 + i],
                    )
                # s3 = m2 (x) s2
                for i in range(5):
                    nc.vector.tensor_scalar_mul(
                        s3[:, i * 25 : i * 25 + 26],
                        s2[:, 0:26],
                        mt[:, b + 10 + i : b + 11 + i],
                    )
                # s4 = m1 (x) s3
                for i in range(5):
                    nc.vector.tensor_scalar_mul(
                        s4[:, i * 125 : i * 125 + 126],
                        s3[:, 0:126],
                        mt[:, b + 5 + i : b + 6 + i],
                    )
                # ot = m0 (x) s4: segments 0-2 on ACT (exact width),
                # segments 3-4 on DVE (padded; 3 stomps 4's first col, 4
                # stomps the pad col — both DVE-internal, in order).
                for i in range(3):
                    nc.scalar.activation(
                        ot[:, i * 625 : (i + 1) * 625],
                        s4[:, 0:625],
                        mybir.ActivationFunctionType.Copy,
                        scale=mt[:, b + i : b + i + 1],
                    )
                for i in range(3, 5):
                    nc.vector.tensor_scalar_mul(
                        ot[:, i * 625 : i * 625 + 626],
                        s4[:, 0:626],
                        mt[:, b + i : b + i + 1],
                    )
                nc.sync.dma_start(out=out[t * 128 : (t + 1) * 128, :], in_=ot[:, 0:R])
    nc.compile()
    return nc


def _pack_inputs(inputs):
    m = [np.asarray(inputs[f"m{j}"], dtype=np.float32) for j in range(5)]
    cat = np.concatenate(m, axis=1)  # (N, 25), col j*5+k = m_j[:, k]
    cat = cat.reshape(N_CORES, NT, 128, 25)
    packed = np.ascontiguousarray(cat.transpose(0, 2, 1, 3).reshape(N_CORES, 128, NT * 25))
    return [{"mcat": packed[c]} for c in range(N_CORES)]


_CACHED_NC = None


def kernel(**inputs) -> np.ndarray:
    global _CACHED_NC
    in_maps = _pack_inputs(inputs)
    if _CACHED_NC is None:
        _CACHED_NC = build_bass()
    res = run_bass_kernel_spmd(_CACHED_NC, in_maps, core_ids=list(range(N_CORES)))
    return np.concatenate([res.results[c]["out"] for c in range(N_CORES)], axis=0)
